# revision 13
# baseline (speedup 1.0000x reference)
"""Gated-RNN (E67H) Trainium2 Bass kernel.

Math (per batch row b, per channel c):
    xp   = silu(x @ W_in.T)                      [T, D]
    P_t  = xp @ W_alpha.T + b_alpha              [T, D]
    v_t  = tanh(xp @ W_x.T + b_v)                [T, D]
    a_t  = sigmoid(P_t + d_alpha * h_{t-1})
    h_t  = a_t * h_{t-1} + (1 - a_t) * v_t
    g_t  = h_t * silu(h_t)
    out  = g @ W_out.T                           [T, D]
    (returns (out, h_T) like the reference)

Sharding: data-parallel over B=8 across the 8 cores (one batch row each),
weights replicated, no collectives.

On-chip strategy: every GEMM runs "transposed" (channels on partitions, time
on the free axis) so the T-sequential scan works on [128, 8] tiles
(D = 1024 = 128 partitions x 8 free blocks).  The scan is rewritten in
u-space (u = d_alpha*h) to keep the per-step chain at 4 ops:
    z = u + P_t (DVE) -> alpha = sigmoid(z) (ACT) -> m = alpha*(u - v~) (DVE)
    -> u' = m + v~ (DVE),  with  d = u - v~  computed on GPSIMD in parallel.
GEMMs use bf16 operands (fp32 accumulation in PSUM) and overlap with the
scan via T-chunking.
"""

import numpy as np

B, T, D = 8, 2048, 1024
P = 128            # partitions
JB = D // P        # 8 channel blocks
TC = 256           # time-chunk
NCH = T // TC      # 8 chunks
NCORES = 8

_cache = {}


def _build():
    import concourse.bass as bass  # noqa: F401
    import concourse.mybir as mybir
    import concourse.tile as tile
    from concourse import bacc

    dt = mybir.dt
    AF = mybir.ActivationFunctionType
    f32 = dt.float32
    bf16 = dt.bfloat16

    nc = bacc.Bacc("TRN2", target_bir_lowering=False, debug=False)

    x_d = nc.dram_tensor("x", [T, D], f32, kind="ExternalInput").ap()
    h0_d = nc.dram_tensor("h0", [D], f32, kind="ExternalInput").ap()
    Win_d = nc.dram_tensor("W_in", [D, D], f32, kind="ExternalInput").ap()
    Wa_d = nc.dram_tensor("W_alpha", [D, D], f32, kind="ExternalInput").ap()
    Wx_d = nc.dram_tensor("W_x", [D, D], f32, kind="ExternalInput").ap()
    Wo_d = nc.dram_tensor("W_out", [D, D], f32, kind="ExternalInput").ap()
    da_d = nc.dram_tensor("d_alpha", [D], f32, kind="ExternalInput").ap()
    ba_d = nc.dram_tensor("b_alpha", [D], f32, kind="ExternalInput").ap()
    bv_d = nc.dram_tensor("b_v", [D], f32, kind="ExternalInput").ap()
    out_d = nc.dram_tensor("out", [T, D], f32, kind="ExternalOutput").ap()
    hN_d = nc.dram_tensor("h_final", [D], f32, kind="ExternalOutput").ap()

    add = mybir.AluOpType.add
    sub = mybir.AluOpType.subtract
    mult = mybir.AluOpType.mult

    with tile.TileContext(nc) as tc:
        with (
            tc.tile_pool(name="wpool", bufs=4) as wpool,
            tc.tile_pool(name="consts", bufs=1) as consts,
            tc.tile_pool(name="pst", bufs=2, space="PSUM") as pstp,
        ):
            # ---- per-channel constant vectors as [128, JB] (c = j*128 + p)
            def load_vec(ap1d, tag):
                t_ = consts.tile([P, JB], f32, tag=tag)
                nc.sync.dma_start(out=t_, in_=ap1d.rearrange("(j p) -> p j", p=P))
                return t_

            da_t = load_vec(da_d, "da")
            ba_t = load_vec(ba_d, "ba")
            bv_t = load_vec(bv_d, "bv")
            h0_t = load_vec(h0_d, "h0")
            invda_t = consts.tile([P, JB], f32)
            nc.vector.reciprocal(invda_t, da_t)
            u0 = consts.tile([P, JB], f32)
            nc.gpsimd.tensor_tensor(u0, h0_t, da_t, op=mult)

            from concourse.masks import make_identity
            ident = consts.tile([P, P], f32)
            make_identity(nc, ident)

            # ---- weights: load fp32, cast to bf16, DMA-xbar transpose to
            #      WT[k-block of d][128, D] layout (lhsT for the GEMMs).
            WTs = {}
            with tc.tile_pool(name="wloadf", bufs=2) as wloadf:
                for name, wap in (("in", Win_d), ("a", Wa_d),
                                  ("x", Wx_d), ("o", Wo_d)):
                    wf = wloadf.tile([P, JB, D], f32)      # [e-block][e, d]
                    nc.sync.dma_start(
                        out=wf, in_=wap.rearrange("(eb e) d -> e eb d", e=P))
                    wt = wpool.tile([P, JB, D], bf16)     # [d-block][d, e]
                    for eb in range(JB):
                        for k in range(JB):
                            tp = pstp.tile([P, P], f32)
                            nc.tensor.transpose(
                                tp, wf[:, eb, P * k:P * (k + 1)], ident)
                            nc.vector.tensor_copy(
                                wt[:, k, P * eb:P * (eb + 1)], tp)
                    WTs[name] = wt

            WinT, WaT, WxT, WoT = WTs["in"], WTs["a"], WTs["x"], WTs["o"]

            with (
                tc.tile_pool(name="xf", bufs=2) as xfp,
                tc.tile_pool(name="xt", bufs=2) as xtp,
                tc.tile_pool(name="xp", bufs=2) as xpp,
                tc.tile_pool(name="pv", bufs=2) as pvp,
                tc.tile_pool(name="uu", bufs=2) as uup,
                tc.tile_pool(name="ep", bufs=2) as epp,
                tc.tile_pool(name="gt", bufs=2) as gtp,
                tc.tile_pool(name="ob", bufs=2) as obp,
                tc.tile_pool(name="sc", bufs=3) as scp,
                tc.tile_pool(name="ps", bufs=4, space="PSUM") as psp,
                tc.tile_pool(name="ps4", bufs=2, space="PSUM") as ps4p,
            ):
                TB = TC // P  # 2 t-row blocks per chunk
                U_prev = None
                carry = []  # (U, hjs) of previous chunk, for epilogue+G4

                for c in range(NCH):
                    t0 = c * TC
                    # -- load x chunk, cast bf16, xbar-transpose to [d, t]
                    xf = xfp.tile([P, TB, D], f32)
                    nc.sync.dma_start(
                        out=xf,
                        in_=x_d[t0:t0 + TC, :].rearrange(
                            "(tb t) d -> t tb d", t=P))
                    xbT = xtp.tile([P, JB, TC], bf16)   # [d-blk k][d, t]
                    for tb in range(TB):
                        for k in range(JB):
                            tp = pstp.tile([P, P], f32)
                            nc.tensor.transpose(
                                tp, xf[:, tb, P * k:P * (k + 1)], ident)
                            nc.vector.tensor_copy(
                                xbT[:, k, P * tb:P * (tb + 1)], tp)

                    # -- G1: xpT = silu(W_in @ x^T)  [e-blk j][e, t] bf16
                    xpT = xpp.tile([P, JB, TC], bf16)
                    for j in range(JB):
                        pt = psp.tile([P, TC], f32)
                        for k in range(JB):
                            nc.tensor.matmul(
                                pt, WinT[:, k, P * j:P * (j + 1)],
                                xbT[:, k, :],
                                start=(k == 0), stop=(k == JB - 1))
                        nc.scalar.activation(xpT[:, j, :], pt, AF.Silu)

                    # -- G2: P_sb[:, t, j] = (W_alpha @ xp^T) + b_alpha
                    P_sb = pvp.tile([P, TC, JB], f32)
                    for j in range(JB):
                        pt = psp.tile([P, TC], f32)
                        for k in range(JB):
                            nc.tensor.matmul(
                                pt, WaT[:, k, P * j:P * (j + 1)],
                                xpT[:, k, :],
                                start=(k == 0), stop=(k == JB - 1))
                        nc.vector.tensor_scalar_add(
                            P_sb[:, :, j], pt, ba_t[:, j:j + 1])

                    # -- G3: V_sb[:, t, j] = d_alpha * tanh((W_x @ xp^T)+b_v)
                    V_sb = pvp.tile([P, TC, JB], f32)
                    for j in range(JB):
                        pt = psp.tile([P, TC], f32)
                        for k in range(JB):
                            nc.tensor.matmul(
                                pt, WxT[:, k, P * j:P * (j + 1)],
                                xpT[:, k, :],
                                start=(k == 0), stop=(k == JB - 1))
                        vt = scp.tile([P, TC], f32)
                        nc.scalar.activation(vt, pt, AF.Tanh,
                                             bias=bv_t[:, j:j + 1])
                        nc.gpsimd.tensor_scalar_mul(
                            V_sb[:, :, j], vt, da_t[:, j:j + 1])

                    # -- epilogue + G4 of the PREVIOUS chunk (overlaps scan c)
                    if carry:
                        _emit_epilogue(nc, mybir, carry[0], WoT, epp, gtp,
                                       obp, ps4p, out_d, hN_d, invda_t,
                                       last=(c == NCH))
                        carry.pop()

                    # -- the sequential scan for chunk c
                    U = uup.tile([P, TC, JB], f32)
                    for t in range(TC):
                        if t == 0:
                            up = u0 if c == 0 else U_prev[:, TC - 1, :]
                        else:
                            up = U[:, t - 1, :]
                        zt = scp.tile([P, JB], f32)
                        nc.gpsimd.tensor_tensor(zt, up, P_sb[:, t, :], op=add)
                        at = scp.tile([P, JB], f32)
                        nc.scalar.activation(at, zt, AF.Sigmoid)
                        dtl = scp.tile([P, JB], f32)
                        nc.vector.tensor_tensor(dtl, up, V_sb[:, t, :], op=sub)
                        mt = scp.tile([P, JB], f32)
                        nc.gpsimd.tensor_tensor(mt, at, dtl, op=mult)
                        nc.gpsimd.tensor_tensor(U[:, t, :], mt, V_sb[:, t, :],
                                                op=add)
                    U_prev = U
                    carry.append((c, U))

                # final chunk's epilogue + G4
                _emit_epilogue(nc, mybir, carry[0], WoT, epp, gtp, obp, ps4p,
                               out_d, hN_d, invda_t, last=True)

    nc.compile()
    return nc


def _emit_epilogue(nc, mybir, carry, WoT, epp, gtp, obp, ps4p, out_d, hN_d,
                   invda_t, last):
    """h = u/d_alpha; g = h*silu(h) (bf16); out = g^T @ W_out^T; DMA out.
    For the last chunk also DMA h_final from the last time column."""
    AF = mybir.ActivationFunctionType
    dt = mybir.dt
    mult = mybir.AluOpType.mult
    c, U = carry
    t0 = c * TC
    TB = TC // P
    gT = gtp.tile([P, JB, TC], dt.bfloat16)
    for j in range(JB):
        hj = epp.tile([P, TC], dt.float32)
        nc.vector.tensor_scalar_mul(hj, U[:, :, j], invda_t[:, j:j + 1])
        sj = epp.tile([P, TC], dt.float32)
        nc.scalar.activation(sj, hj, AF.Silu)
        nc.gpsimd.tensor_tensor(gT[:, j, :], hj, sj, op=mult)
        if last:
            nc.sync.dma_start(out=hN_d[P * j:P * (j + 1)],
                              in_=hj[:, TC - 1:TC])
    ob = obp.tile([P, TB, D], dt.float32)
    for m in range(TB):
        for n in range(2):
            pt = ps4p.tile([P, 512], dt.float32)
            for j in range(JB):
                nc.tensor.matmul(
                    pt, gT[:, j, P * m:P * (m + 1)],
                    WoT[:, j, 512 * n:512 * (n + 1)],
                    start=(j == 0), stop=(j == JB - 1))
            nc.vector.tensor_copy(ob[:, m, 512 * n:512 * (n + 1)], pt)
    nc.sync.dma_start(
        out=out_d[t0:t0 + TC, :].rearrange("(tb t) d -> t tb d", t=P),
        in_=ob)


def _get_nc():
    if "nc" not in _cache:
        _cache["nc"] = _build()
    return _cache["nc"]


def kernel(x, h0, W_in, W_alpha, d_alpha, b_alpha, W_x, b_v, W_out):
    from concourse.bass_utils import run_bass_kernel_spmd

    nc = _get_nc()
    x = np.asarray(x, dtype=np.float32)
    h0 = np.asarray(h0, dtype=np.float32)
    shared = {
        "W_in": np.asarray(W_in, np.float32),
        "W_alpha": np.asarray(W_alpha, np.float32),
        "W_x": np.asarray(W_x, np.float32),
        "W_out": np.asarray(W_out, np.float32),
        "d_alpha": np.asarray(d_alpha, np.float32),
        "b_alpha": np.asarray(b_alpha, np.float32),
        "b_v": np.asarray(b_v, np.float32),
    }
    in_maps = [dict(shared, x=x[b], h0=h0[b]) for b in range(NCORES)]
    res = run_bass_kernel_spmd(nc, in_maps, core_ids=list(range(NCORES)))
    out = np.stack([res.results[b]["out"] for b in range(NCORES)])
    hN = np.stack([res.results[b]["h_final"] for b in range(NCORES)])
    return out, hN
